# revision 33
# baseline (speedup 1.0000x reference)
"""Trainium2 Bass kernel for nn_BiDGNBlock (moe_routing).

Strategy: data-parallel over batch across 8 NeuronCores (no collectives —
measured collective floors ~10-25us each beat any sharded-expert scheme).
Each core computes one batch element end-to-end.

Optimizations over the 70.6us fp16 baseline (trace-driven):
  - Expert table shipped as float8e3 (e3m4) scaled by 128: halves the
    dominant 8.4MB->4.2MB DMA; the PE runs mixed fp16(stationary) x
    f8e3(moving) matmuls at bf16 rate. LN2's scale-invariance absorbs the
    128 (its sqrt bias eps is pre-scaled by 128^2, keeping LN exact).
    Graded rel err 6.6e-3 (gate 2e-2), matching the host simulation.
  - Router row-norms dropped entirely: top-2 picks are invariant to a
    positive per-row scale of sim, and the gate (softmax over top-2,
    summed) is exactly 1.
  - No ACT-table thrash: only Exp (softmax, with accum_out rowsum) and
    Sqrt (both LNs) are used, one table switch total; all reciprocals via
    the exact DVE iterative divide on [*,1] columns. (Ln/Exp-based rsqrt
    and the blocked Rsqrt/Reciprocal tables cost 1.28us per switch.)
  - Fully transposed attention/LN1 ([feature-part, channel]): proj output,
    LN1 stats (PE ones-matmuls, column form), residual (host-precombined
    (beta+x).T) all avoid transposes on the critical path; the natural-
    space transposes for the final residual run inside the expert phase.
  - Masked activations built as fp32 pairs: oAll fp16 [p, side, C, kt]
    with kt innermost, mask-multiply on the fp32 bitcast view (half the
    DVE elements). Masks for experts 0-7 built inline (batched is_equal
    against the PE-replicated top-2 rows) to cover the fp16 DRAM
    mask-replication round-trip; experts 8-63 read replicated rows.
  - Expert stage: first groups sized 2/2/4 so matmuls start ~1us earlier;
    one interleaved (side,C) bias matmul seeds the PSUM accumulation.
  - Warm-filler matmuls placed in the real dependency stalls (softmax,
    LN1 coef chain, top-k, mask round-trip) keep the PE HAM clock-gate at
    K=8/8 into the 128-matmul expert phase (fp8 spacing ~109ns warm).
  - Top-k critical path trimmed: the router bias-add runs on ACT (per-
    partition bias AP), the oAll fp16 copy is deferred past the top-k ops,
    and LN2's -mean*rstd folds into one fused tensor_scalar.
Measured: ~59-61us HW exec typical (best 58.5us; HAM/DMA phase jitter can
add up to ~15us run-to-run), rel err 6.56e-3.
"""

import sys
import numpy as np

sys.path.insert(0, "/opt/trn_rl_repo")

N_CORES = 8
B, C, T = 8, 64, 256
EXP = 32
KT = T // 128  # 2 k-tiles over the feature dim
EG = 4         # experts per grouped mask-multiply
INL = 8        # experts masked inline (cover the round-trip latency)
WE_SCALE = 128.0

_CACHE: dict = {}

# fp32 blob layouts: (name, partitions, shape). cols = prod(shape[1:]).
BLOB_A1_SPEC = [
    ("xtl", 128, (128, KT, C)), ("wqt", 128, (128, KT, T)),
    ("bqp", 128, (128, KT)),
]
BLOB_A1B_SPEC = [
    ("xtr", 128, (128, KT, C)), ("wkt", 128, (128, KT, T)),
    ("bkp", 128, (128, KT)),
]
BLOB_A2_SPEC = [
    ("wvt", 128, (128, KT, T)), ("bv", 64, (64, T)),
]
BLOB_B_SPEC = [
    ("wpt", 128, (128, KT, T)), ("wrt", 128, (128, 2 * KT, EXP)),
    ("ident", 128, (128, 128)), ("sel", 2, (2, 2, 128)),
    ("bpp", 128, (128, KT)),
    ("agp", 128, (128, 2, KT)),          # [p, side, kt] LN1 gamma per-partition
    ("ablx", 128, (128, KT, 2, C)),      # (beta + x).T  [p, kt, side, C]
    ("mgall", 128, (128, T)),            # LN2 gamma rows, (c, side) interleaved
    ("mball", 128, (128, T)),            # LN2 beta rows, (c, side) interleaved
    ("cent", 32, (32, C)),
    ("brp", 32, (32, 1)),
    ("eiota", 64, (64, 1)),
    ("identh", 64, (64, C // 2)),
    ("eior8", 128, (128, INL)),          # iota 0..INL-1 on all partitions
    ("onesc", 128, (128, 1)),            # ones column for stats matmuls
    ("onesr", 1, (1, 128)),              # ones row for coef replication
]


def _blob_layout():
    off = {}
    na1 = 0
    for name, parts, shape in BLOB_A1_SPEC:
        cols = int(np.prod(shape[1:]))
        off[name] = (na1, parts, shape)
        na1 += cols
    na1b = 0
    for name, parts, shape in BLOB_A1B_SPEC:
        cols = int(np.prod(shape[1:]))
        off[name] = (na1b, parts, shape)
        na1b += cols
    na2 = 0
    for name, parts, shape in BLOB_A2_SPEC:
        cols = int(np.prod(shape[1:]))
        off[name] = (na2, parts, shape)
        na2 += cols
    nb = 0
    for name, parts, shape in BLOB_B_SPEC:
        cols = int(np.prod(shape[1:]))
        off[name] = (nb, parts, shape)
        nb += cols
    return off, na1, na1b, na2, nb


BLOB_OFF, NA1_COLS, NA1B_COLS, NA2_COLS, NB_COLS = _blob_layout()


def _build():
    import concourse.bass as bass
    import concourse.mybir as mybir
    import concourse.tile as tile
    from concourse import bacc
    from contextlib import ExitStack

    dt = mybir.dt
    f32, f16, f8 = dt.float32, dt.float16, dt.float8e3
    AF = mybir.ActivationFunctionType
    OP = mybir.AluOpType

    nc = bacc.Bacc("TRN2", target_bir_lowering=False, debug=False,
                   num_devices=N_CORES)

    def inp(name, shape, d=f32):
        return nc.dram_tensor(name, list(shape), d, kind="ExternalInput")

    blobA1_d = inp("blobA1", (128, NA1_COLS))
    blobA1b_d = inp("blobA1b", (128, NA1B_COLS))
    blobA2_d = inp("blobA2", (128, NA2_COLS))
    blobB_d = inp("blobB", (128, NB_COLS))
    weh_d = inp("weh", (128, C, KT, T), f8)   # We[e].T tiled [p, e, kt, u], x128
    beh_d = inp("beh", (C, T), f16)           # be natural fp16, x128

    o2_d = nc.dram_tensor("o2", [128, T], f32, kind="ExternalOutput")

    with tile.TileContext(nc) as tc, ExitStack() as ctx:
        cst = ctx.enter_context(tc.tile_pool(name="cst", bufs=1))
        wk = ctx.enter_context(tc.tile_pool(name="wk", bufs=2))
        sm = ctx.enter_context(tc.tile_pool(name="sm", bufs=2))
        asc_p = ctx.enter_context(tc.tile_pool(name="asc", bufs=4))
        ps = ctx.enter_context(tc.tile_pool(name="ps", bufs=4, space="PSUM"))
        ps_moe_p = ctx.enter_context(tc.tile_pool(name="psmoe", bufs=1, space="PSUM"))
        warm_p = ctx.enter_context(tc.tile_pool(name="warm", bufs=1, space="PSUM"))

        # ---- warm-up source: very first DVE op, so the PE warm-up can
        # start the moment the start barrier clears ----
        wsrc = cst.tile([128, 512], f16, tag="wsrc")
        nc.vector.memset(wsrc, 0.5)

        # ---- loads: attention-critical first ----
        blobA1 = cst.tile([128, NA1_COLS], f32, tag="blobA1")
        nc.sync.dma_start(out=blobA1, in_=blobA1_d.ap())
        blobA1b = cst.tile([128, NA1B_COLS], f32, tag="blobA1b")
        nc.scalar.dma_start(out=blobA1b, in_=blobA1b_d.ap())
        blobA2 = cst.tile([128, NA2_COLS], f32, tag="blobA2")
        nc.sync.dma_start(out=blobA2, in_=blobA2_d.ap())
        blobB = cst.tile([128, NB_COLS], f32, tag="blobB")
        nc.sync.dma_start(out=blobB, in_=blobB_d.ap())
        we_sb = cst.tile([128, C, KT, T], f8, tag="weh")
        wea = weh_d.ap()
        for ch in range(4):
            nc.sync.dma_start(out=we_sb[:, ch * 16:(ch + 1) * 16],
                              in_=wea[:, ch * 16:(ch + 1) * 16])
        beh = cst.tile([C, T], f16, tag="beh")
        nc.scalar.dma_start(out=beh, in_=beh_d.ap())

        def bview(blob, name):
            off, parts, shape = BLOB_OFF[name]
            cols = 1
            for s in shape[1:]:
                cols *= s
            v = blob[0:parts, off:off + cols]
            if len(shape) == 3:
                v = v.rearrange("p (a b) -> p a b", a=shape[1])
            elif len(shape) == 4:
                v = v.rearrange("p (a b c) -> p a b c", a=shape[1], b=shape[2])
            return v

        xtl = bview(blobA1, "xtl")
        wqt = bview(blobA1, "wqt")
        bqp = bview(blobA1, "bqp")
        xtr = bview(blobA1b, "xtr")
        wkt = bview(blobA1b, "wkt")
        bkp = bview(blobA1b, "bkp")
        wvt = bview(blobA2, "wvt")
        bv = bview(blobA2, "bv")
        wpt = bview(blobB, "wpt")
        wrt = bview(blobB, "wrt")
        ident = bview(blobB, "ident")
        sel = bview(blobB, "sel")
        bpp = bview(blobB, "bpp")
        agp = bview(blobB, "agp")
        ablx = bview(blobB, "ablx")
        mgall = bview(blobB, "mgall")
        mball = bview(blobB, "mball")
        cent = bview(blobB, "cent")
        brp = bview(blobB, "brp")
        eiota = bview(blobB, "eiota")
        identh_f32 = bview(blobB, "identh")
        identh = identh_f32.bitcast(mybir.dt.float16)
        eior8 = bview(blobB, "eior8")
        onesc = bview(blobB, "onesc")
        onesr = bview(blobB, "onesr")

        eps1_t = cst.tile([128, 1], f32, tag="eps1")
        nc.vector.memset(eps1_t, 1e-5)
        eps2_t = cst.tile([128, 1], f32, tag="eps2")
        nc.vector.memset(eps2_t, 1e-5 * WE_SCALE * WE_SCALE)
        # ACT table preloads
        wact = cst.tile([1, 32], f32, tag="wact")
        nc.vector.memset(wact, 1.0)
        nc.scalar.activation(out=wact, in_=wact, func=AF.Sqrt)
        nc.scalar.activation(out=wact, in_=wact, func=AF.Exp)

        def warm(n):
            pwx = warm_p.tile([128, 512], f32, tag="warm")
            for _ in range(n):
                nc.tensor.matmul(pwx, wsrc[:, 0:128], wsrc,
                                 start=True, stop=True, skip_group_check=True)

        # ---- attention ----
        # xdt early: DVE computes it while PE runs q/k
        xdt = wk.tile([128, KT, C], f32, tag="xdt")
        nc.vector.tensor_sub(xdt, xtl, xtr)
        # q.T (prescaled by 1/16 incl. bias), k.T  [u(128), kt, c]
        # bias adds on the ACT engine (DVE stays free)
        qt = wk.tile([128, KT, C], f32, tag="qt")
        ktl = wk.tile([128, KT, C], f32, tag="ktl")
        for (src, w, bias, dst, s2) in [(xtl, wqt, bqp, qt, 1.0 / 16.0),
                                        (xtr, wkt, bkp, ktl, 1.0)]:
            for ut in range(KT):
                p = ps.tile([128, C], f32, tag="ps")
                for kt in range(KT):
                    nc.tensor.matmul(p, w[:, kt, ut * 128:(ut + 1) * 128],
                                     src[:, kt], start=(kt == 0), stop=(kt == KT - 1))
                nc.scalar.activation(out=dst[:, ut], in_=p, func=AF.Identity,
                                     bias=bias[:, ut:ut + 1], scale=s2)

        # ---- energy + softmax (no max-subtraction; |e/16| < ~1) ----
        pe_ = ps.tile([C, C], f32, tag="ps")
        for ut in range(KT):
            nc.tensor.matmul(pe_, qt[:, ut], ktl[:, ut],
                             start=(ut == 0), stop=(ut == KT - 1))
        warm(2)
        attn = wk.tile([C, C], f32, tag="attn")
        rowsum = sm.tile([C, 1], f32, tag="rowsum")
        nc.scalar.activation(out=attn, in_=pe_, func=AF.Exp, accum_out=rowsum)

        # ---- v = (x_l - x_r) @ Wv.T + bv: overlaps the softmax chain ----
        pv = ps.tile([C, T], f32, tag="ps")
        for kt in range(KT):
            nc.tensor.matmul(pv, xdt[:, kt], wvt[:, kt],
                             start=(kt == 0), stop=(kt == KT - 1))
        v_sb = wk.tile([C, T], f32, tag="v")
        nc.vector.tensor_tensor(out=v_sb, in0=pv, in1=bv, op=OP.add)

        rinv = sm.tile([C, 1], f32, tag="rinv")
        nc.vector.reciprocal(rinv, rowsum)
        nc.vector.tensor_scalar_mul(attn, attn, rinv)

        # ---- attn.T ----
        pat = ps.tile([C, C], f32, tag="ps")
        nc.tensor.transpose(pat, attn, ident[0:C, 0:C])
        attnT = wk.tile([C, C], f32, tag="attnT")
        nc.vector.tensor_copy(attnT, pat)

        # ---- out_l.T / out_r.T  [u, c] ----
        oLT = wk.tile([128, KT, C], f32, tag="oLT")
        oRT = wk.tile([128, KT, C], f32, tag="oRT")
        for ut in range(KT):
            pl = ps.tile([128, C], f32, tag="ps")
            nc.tensor.matmul(pl, v_sb[:, ut * 128:(ut + 1) * 128], attnT,
                             start=True, stop=True)
            nc.vector.tensor_copy(oLT[:, ut], pl)
        warm(3)
        for ut in range(KT):
            pr = ps.tile([128, C], f32, tag="ps")
            nc.tensor.matmul(pr, v_sb[:, ut * 128:(ut + 1) * 128], attn,
                             start=True, stop=True)
            nc.scalar.activation(out=oRT[:, ut], in_=pr, func=AF.Identity)

        # ---- proj (transposed): pT [p, kt, side, C] = (Wp @ out.T) + bp ----
        pT = wk.tile([128, KT, 2, C], f32, tag="pT")
        for s, oT in ((0, oLT), (1, oRT)):
            for ut in range(KT):
                pp = ps.tile([128, C], f32, tag="ps")
                for kt in range(KT):
                    nc.tensor.matmul(pp, wpt[:, kt, ut * 128:(ut + 1) * 128],
                                     oT[:, kt], start=(kt == 0), stop=(kt == KT - 1))
                if s == 0:
                    nc.scalar.activation(out=pT[:, ut, s], in_=pp,
                                         func=AF.Identity,
                                         bias=bpp[:, ut:ut + 1])
                else:
                    nc.vector.tensor_scalar(out=pT[:, ut, s], in0=pp,
                                            scalar1=bpp[:, ut:ut + 1],
                                            scalar2=None, op0=OP.add)

        # ---- LN1 in transposed space: per-column stats via PE ones-matmuls --
        sq = wk.tile([128, KT, 2, C], f32, tag="sq")
        nc.vector.tensor_mul(sq, pT, pT)
        # column-form stats: psum [128(s,c), 2] via pT/sq as stationary
        pstat = ps.tile([128, 2], f32, tag="ps")
        for kt in range(KT):
            nc.tensor.matmul(pstat[:, 0:1], pT[:, kt], onesc,
                             start=(kt == 0), stop=(kt == KT - 1))
        for kt in range(KT):
            nc.tensor.matmul(pstat[:, 1:2], sq[:, kt], onesc,
                             start=(kt == 0), stop=(kt == KT - 1))
        warm(4)
        # coefs: a = rstd, b = mean * rstd   (LN = pT*a - b, then *g + ablx)
        msv = sm.tile([128, 2], f32, tag="msv")
        nc.vector.tensor_scalar(out=msv, in0=pstat, scalar1=1.0 / T,
                                scalar2=None, op0=OP.mult)
        m1 = msv[:, 0:1]
        var = sm.tile([128, 1], f32, tag="var")
        nc.vector.tensor_mul(var, m1, m1)
        nc.vector.tensor_sub(var, msv[:, 1:2], var)
        coefc = sm.tile([128, 2], f32, tag="coefc")
        nc.scalar.activation(out=coefc[:, 0:1], in_=var, func=AF.Sqrt,
                             bias=eps1_t)
        nc.vector.reciprocal(coefc[:, 0:1], coefc[:, 0:1])
        nc.vector.tensor_mul(coefc[:, 1:2], m1, coefc[:, 0:1])
        # replicate the two coef columns to all partitions: transpose + 2 MMs
        ptc = ps.tile([2, 128], f32, tag="ps")
        nc.tensor.transpose(ptc, coefc, ident)
        coefr = sm.tile([2, 128], f32, tag="coefr")
        nc.scalar.activation(out=coefr, in_=ptc, func=AF.Identity)
        prep = ps.tile([128, 2, 128], f32, tag="ps")
        for j in range(2):
            nc.tensor.matmul(prep[:, j], sel[:, j], coefr,
                             start=True, stop=True)
        warm(3)
        aB = prep[:, 0].rearrange("p (s c) -> p s c", s=2).unsqueeze(1) \
            .broadcast_to((128, KT, 2, C))
        bB = prep[:, 1].rearrange("p (s c) -> p s c", s=2).unsqueeze(1) \
            .broadcast_to((128, KT, 2, C))
        nc.vector.tensor_mul(pT, pT, aB)
        nc.vector.tensor_sub(pT, pT, bB)
        for s in range(2):
            for kt in range(KT):
                if s == 0:
                    nc.scalar.activation(out=pT[:, kt, s], in_=pT[:, kt, s],
                                         func=AF.Identity,
                                         scale=agp[:, s, kt:kt + 1])
                else:
                    nc.vector.tensor_scalar(out=pT[:, kt, s], in0=pT[:, kt, s],
                                            scalar1=agp[:, s, kt:kt + 1],
                                            scalar2=None, op0=OP.mult)
        nc.vector.tensor_add(pT, pT, ablx)
        # pT now holds OUT.T (LN1 output + residual), both sides.

        # ---- router (row-norms dropped; picks invariant to row scale) ----
        pxp = ps.tile([EXP, C], f32, tag="ps")
        j = 0
        for s in range(2):
            for kt in range(KT):
                nc.tensor.matmul(pxp, wrt[:, j], pT[:, kt, s],
                                 start=(j == 0), stop=(j == 2 * KT - 1))
                j += 1
        xpT = wk.tile([EXP, C], f32, tag="xpT")
        nc.scalar.activation(out=xpT, in_=pxp, func=AF.Identity, bias=brp)
        psim = ps.tile([C, C], f32, tag="ps")
        nc.tensor.matmul(psim, xpT, cent, start=True, stop=True)

        warm(3)  # PE filler while DVE runs top-k

        mx8 = sm.tile([C, 8], f32, tag="mx8")
        nc.vector.max(out=mx8, in_=psim)
        idx8 = sm.tile([C, 8], mybir.dt.uint32, tag="idx8")
        nc.vector.max_index(out=idx8, in_max=mx8, in_values=psim)

        # ---- replicate topi rows across partitions: one matmul per k with a
        # stride-0 stationary (topih column broadcast over 128 cols) ----
        topih = sm.tile([C, 2], f16, tag="topih")
        nc.vector.tensor_copy(topih, idx8[:, 0:2])

        # fp16 copy, kt innermost: oAll [p, side, C, kt]
        oAll = wk.tile([128, 2, C, KT], f16, tag="oAll")
        nc.vector.tensor_copy(oAll, pT[:].transpose([0, 2, 3, 1]))
        ttrep_ps = []
        for k in range(2):
            pr = ps.tile([128, C], f32, tag="ps")
            nc.tensor.matmul(pr,
                             topih[:, k:k + 1].broadcast_to((C, 128)),
                             identh, start=True, stop=True)
            ttrep_ps.append(pr)

        warm(2)

        # ---- RT[e, c] one-hot-sum mask matrix ----
        RT = wk.tile([C, C], f32, tag="RT")
        RT1 = sm.tile([C, C], f32, tag="RT1")
        nc.vector.tensor_scalar(out=RT, in0=ttrep_ps[0][0:C], scalar1=eiota,
                                scalar2=None, op0=OP.is_equal)
        nc.vector.tensor_scalar(out=RT1, in0=ttrep_ps[1][0:C], scalar1=eiota,
                                scalar2=None, op0=OP.is_equal)
        RTh = wk.tile([C, C], f16, tag="RTh")
        nc.vector.tensor_tensor(out=RTh, in0=RT, in1=RT1, op=OP.add)
        # bias-matmul stationary: RT duplicated to (side, c) columns, fp16
        RTb = wk.tile([C, 2, C], f16, tag="RTb")
        nc.vector.tensor_copy(RTb, RTh.unsqueeze(1).broadcast_to((C, 2, C)))

        # ---- replicate RTh rows across partitions via a DRAM round-trip
        # (HWDGE write on the scalar ring; SWDGE replicated reads) ----
        dram = ctx.enter_context(tc.tile_pool(name="dram", bufs=1, space="DRAM"))
        rtd = dram.tile([C, C], f16)
        nc.scalar.dma_start(out=rtd[:], in_=RTh)
        rrep = wk.tile([128, C, C], f16, tag="rrep")
        rsrc = rtd[:]
        for (c0, c1) in ((INL, 12), (12, 24), (24, 40), (40, C)):
            src_ap = bass.AP(tensor=rsrc.tensor, offset=rsrc.offset + c0 * C,
                             ap=[[0, 128], [C, c1 - c0], [1, C]])
            nc.scalar.dma_start(out=rrep[:, c0:c1], in_=src_ap)

        # ---- inline masks for experts 0..INL-1 (batched is_equal, in
        # halves so the first expert matmuls start sooner) ----
        allm = wk.tile([128, INL, C], f32, tag="allm")
        allm1 = wk.tile([128, INL, C], f32, tag="allm1")
        H = INL // 2
        for h0 in (0, H):
            hs = slice(h0, h0 + H)
            nc.vector.tensor_tensor(
                out=allm[:, hs],
                in0=ttrep_ps[0].unsqueeze(1).broadcast_to((128, H, C)),
                in1=eior8[:, hs].unsqueeze(2).broadcast_to((128, H, C)),
                op=OP.is_equal)
            nc.vector.tensor_tensor(
                out=allm1[:, hs],
                in0=ttrep_ps[1].unsqueeze(1).broadcast_to((128, H, C)),
                in1=eior8[:, hs].unsqueeze(2).broadcast_to((128, H, C)),
                op=OP.is_equal)
            nc.vector.tensor_add(allm[:, hs], allm[:, hs], allm1[:, hs])

        # ---- expert stage ----
        outnat = wk.tile([128, T], f32, tag="outnat")
        ps_moe = ps_moe_p.tile([128, T], f32, tag="psmoe")
        nc.tensor.matmul(ps_moe, RTb, beh,
                         start=True, stop=False, skip_group_check=True)
        warm(5)
        oPair = oAll[:].bitcast(mybir.dt.float32).squeeze(3)  # [p, side, C]
        groups = [(0, 2), (2, 4), (4, 8)] + \
            [(e, e + EG) for e in range(INL, C, EG)]
        for (e0, e1) in groups:
            ng = e1 - e0
            asch = asc_p.tile([128, ng, 2, C, KT], f16, tag=f"asch{ng}")
            aschP = asch[:].bitcast(mybir.dt.float32).squeeze(4)
            in0 = oPair.unsqueeze(1).broadcast_to((128, ng, 2, C))
            if e0 < INL:
                in1 = allm[:, e0:e1].unsqueeze(2) \
                    .broadcast_to((128, ng, 2, C))
            else:
                in1 = rrep[:, e0:e1].unsqueeze(2) \
                    .broadcast_to((128, ng, 2, C))
            if e0 == 16:
                for s in range(2):
                    for kt in range(KT):
                        ptn = ps.tile([C, 128], f32, tag="ps")
                        nc.tensor.transpose(ptn, pT[:, kt, s], ident)
                        nc.vector.tensor_copy(
                            outnat[s * C:(s + 1) * C,
                                   kt * 128:(kt + 1) * 128], ptn)
            nc.vector.tensor_tensor(out=aschP, in0=in0, in1=in1, op=OP.mult)
            for i in range(ng):
                for kt in range(KT):
                    st = asch[:, i, :, :, kt]
                    nc.tensor.matmul(ps_moe, st, we_sb[:, e0 + i, kt],
                                     start=False,
                                     stop=(e1 >= C and i == ng - 1
                                           and kt == KT - 1),
                                     skip_group_check=True)

        # ---- final LN2 + residual, both sides at once ----
        obl = wk.tile([128, T], f32, tag="obl")
        nc.vector.tensor_add(obl, outnat, mball)
        stats = sm.tile([128, 6], f32, tag="stats2")
        nc.vector.bn_stats(out=stats, in_=ps_moe)
        mv = sm.tile([128, 2], f32, tag="mv2")
        nc.vector.bn_aggr(out=mv, in_=stats)
        rstd2 = sm.tile([128, 1], f32, tag="rstd2")
        nc.scalar.activation(out=rstd2, in_=mv[:, 1:2], func=AF.Sqrt,
                             bias=eps2_t)
        nc.vector.reciprocal(rstd2, rstd2)
        nb2 = sm.tile([128, 1], f32, tag="nb2")
        nc.vector.tensor_scalar(out=nb2, in0=mv[:, 0:1], scalar1=rstd2,
                                scalar2=-1.0, op0=OP.mult, op1=OP.mult)
        o2 = wk.tile([128, T], f32, tag="o2")
        o2a = o2_d.ap()
        for h in range(2):
            cs = slice(h * 128, (h + 1) * 128)
            nc.scalar.activation(out=o2[:, cs], in_=ps_moe[:, cs],
                                 func=AF.Identity, scale=rstd2, bias=nb2)
            nc.vector.tensor_tensor(out=o2[:, cs], in0=o2[:, cs],
                                    in1=mgall[:, cs], op=OP.mult)
            nc.vector.tensor_tensor(out=o2[:, cs], in0=o2[:, cs],
                                    in1=obl[:, cs], op=OP.add)
            nc.sync.dma_start(out=o2a[:, cs], in_=o2[:, cs])

    nc.compile()
    return nc


def _tile_t(w):
    # (T_in, N) -> [128, T_in//128, N] partition-tiled
    t_in, n = w.shape
    return np.ascontiguousarray(w.reshape(t_in // 128, 128, n).transpose(1, 0, 2))


def _interleave_rows(a, b):
    # [C, T], [C, T] -> [2C, T] with rows (c, side)-interleaved
    out = np.empty((2 * C, a.shape[1]), a.dtype)
    out[0::2] = a
    out[1::2] = b
    return out


def _prep_in_maps(inputs):
    f = np.float32
    import ml_dtypes
    x_l, x_r = inputs["x_l"], inputs["x_r"]

    cen = np.asarray(inputs["centers"], f)
    cenn = cen / np.maximum(np.linalg.norm(cen, axis=-1, keepdims=True), 1e-12)
    sel = np.zeros((2, 2, 128), f)
    sel[0, 0, :] = 1.0
    sel[1, 1, :] = 1.0

    def perp(b):  # (T,) -> [128, KT] per-partition layout
        return np.asarray(b, f).reshape(KT, 128).T

    ag = np.stack([np.asarray(inputs["ag_l"], f), np.asarray(inputs["ag_r"], f)])
    agp = ag.reshape(2, KT, 128).transpose(2, 0, 1)  # [p, side, kt]

    mgall = np.concatenate([
        np.repeat(np.asarray(inputs["mg_l"], f).reshape(1, T), C, axis=0),
        np.repeat(np.asarray(inputs["mg_r"], f).reshape(1, T), C, axis=0)])
    mball = np.concatenate([
        np.repeat(np.asarray(inputs["mb_l"], f).reshape(1, T), C, axis=0),
        np.repeat(np.asarray(inputs["mb_r"], f).reshape(1, T), C, axis=0)])

    arrs = {
        "wqt": _tile_t(np.asarray(inputs["Wq"], f).T),
        "wkt": _tile_t(np.asarray(inputs["Wk"], f).T),
        "wvt": _tile_t(np.asarray(inputs["Wv"], f).T),
        "wpt": _tile_t(np.asarray(inputs["Wp"], f).T),
        "bqp": perp(np.asarray(inputs["bq"], f) / 16.0),
        "bkp": perp(inputs["bk"]),
        "bpp": perp(inputs["bp"]),
        "agp": agp,
        "wrt": _tile_t(np.asarray(inputs["Wr"], f).T),
        "brp": np.asarray(inputs["br"], f).reshape(EXP, 1),
        "cent": np.ascontiguousarray(cenn.T),
        "ident": np.eye(128, dtype=f),
        "eiota": np.arange(C, dtype=f).reshape(C, 1),
        "identh": np.eye(C, dtype=np.float16).view(f),
        "eior8": np.repeat(np.arange(INL, dtype=f).reshape(1, INL), 128, axis=0),
        "onesc": np.ones((128, 1), f),
        "onesr": np.ones((1, 128), f),
        "sel": sel,
        "bv": np.repeat(np.asarray(inputs["bv"], f).reshape(1, T), C, axis=0),
        "mgall": mgall, "mball": mball,
    }
    We = np.asarray(inputs["We"], f)
    WeTh = np.ascontiguousarray(
        (We * WE_SCALE).transpose(0, 2, 1).reshape(C, KT, 128, T)
        .transpose(2, 0, 1, 3)
    ).astype(ml_dtypes.float8_e3m4)
    beh = (np.asarray(inputs["be"], f) * WE_SCALE).astype(np.float16)

    def pack(spec, ncols, extra):
        blob = np.zeros((128, ncols), f)
        for name, parts, shape in spec:
            off, _, _ = BLOB_OFF[name]
            cols = int(np.prod(shape[1:]))
            a = extra[name] if name in extra else arrs[name]
            blob[0:parts, off:off + cols] = np.asarray(a, f).reshape(parts, cols)
        return blob

    ab_l = np.asarray(inputs["ab_l"], f)
    ab_r = np.asarray(inputs["ab_r"], f)
    in_maps = []
    for b in range(N_CORES):
        xlb = np.asarray(x_l[b], f)
        xrb = np.asarray(x_r[b], f)
        xtl = _tile_t(np.ascontiguousarray(xlb.T))
        xtr = _tile_t(np.ascontiguousarray(xrb.T))
        blobA1 = pack(BLOB_A1_SPEC, NA1_COLS, {"xtl": xtl})
        blobA1b = pack(BLOB_A1B_SPEC, NA1B_COLS, {"xtr": xtr})
        blobA2 = pack(BLOB_A2_SPEC, NA2_COLS, {})
        # (beta + x).T tiled: [p, kt, side, C]
        ablx = np.stack([_tile_t(np.ascontiguousarray((ab_l + xlb).T)),
                         _tile_t(np.ascontiguousarray((ab_r + xrb).T))], axis=2)
        blobB = pack(BLOB_B_SPEC, NB_COLS, {"ablx": ablx})
        in_maps.append({"blobA1": blobA1, "blobA1b": blobA1b,
                        "blobA2": blobA2, "blobB": blobB,
                        "weh": WeTh, "beh": beh})
    return in_maps


def kernel(**inputs) -> np.ndarray:
    from concourse.bass_utils import run_bass_kernel_spmd

    if "nc" not in _CACHE:
        _CACHE["nc"] = _build()
    nc = _CACHE["nc"]
    in_maps = _prep_in_maps(inputs)
    res = run_bass_kernel_spmd(nc, in_maps, list(range(N_CORES)))
    _CACHE["exec_time_ns"] = res.exec_time_ns
    o2 = np.stack([res.results[b]["o2"] for b in range(N_CORES)])  # [B,128,T]
    out_l2 = o2[:, 0:C]
    out_r2 = o2[:, C:2 * C]
    return np.stack([out_l2, out_r2]).astype(np.float32)


# revision 34
# speedup vs baseline: 1.0080x; 1.0080x over previous
"""Trainium2 Bass kernel for nn_BiDGNBlock (moe_routing).

Strategy: data-parallel over batch across 8 NeuronCores (no collectives —
measured collective floors ~10-25us each beat any sharded-expert scheme).
Each core computes one batch element end-to-end.

Optimizations over the 70.6us fp16 baseline (trace-driven):
  - Expert table shipped as float8e3 (e3m4) scaled by 128: halves the
    dominant 8.4MB->4.2MB DMA; the PE runs mixed fp16(stationary) x
    f8e3(moving) matmuls at bf16 rate. LN2's scale-invariance absorbs the
    128 (its sqrt bias eps is pre-scaled by 128^2, keeping LN exact).
    Graded rel err 6.6e-3 (gate 2e-2), matching the host simulation.
  - Router row-norms dropped entirely: top-2 picks are invariant to a
    positive per-row scale of sim, and the gate (softmax over top-2,
    summed) is exactly 1.
  - No ACT-table thrash: only Exp (softmax, with accum_out rowsum) and
    Sqrt (both LNs) are used, one table switch total; all reciprocals via
    the exact DVE iterative divide on [*,1] columns. (Ln/Exp-based rsqrt
    and the blocked Rsqrt/Reciprocal tables cost 1.28us per switch.)
  - Fully transposed attention/LN1 ([feature-part, channel]): proj output,
    LN1 stats (PE ones-matmuls, column form), residual (host-precombined
    (beta+x).T) all avoid transposes on the critical path; the natural-
    space transposes for the final residual run inside the expert phase.
  - Masked activations built as fp32 pairs: oAll fp16 [p, side, C, kt]
    with kt innermost, mask-multiply on the fp32 bitcast view (half the
    DVE elements). Masks for experts 0-7 built inline (batched is_equal
    against the PE-replicated top-2 rows) to cover the fp16 DRAM
    mask-replication round-trip; experts 8-63 read replicated rows.
  - Expert stage: first groups sized 2/2/4 so matmuls start ~1us earlier;
    one interleaved (side,C) bias matmul seeds the PSUM accumulation.
  - Warm-filler matmuls placed in the real dependency stalls (softmax,
    LN1 coef chain, top-k, mask round-trip) keep the PE HAM clock-gate at
    K=8/8 into the 128-matmul expert phase (fp8 spacing ~109ns warm).
  - Top-k critical path trimmed: the router bias-add runs on ACT (per-
    partition bias AP), the oAll fp16 copy is deferred past the top-k ops,
    and LN2's -mean*rstd folds into one fused tensor_scalar.
Measured: ~59-61us HW exec typical (best 58.5us; HAM/DMA phase jitter can
add up to ~15us run-to-run), rel err 6.56e-3.
"""

import sys
import numpy as np

sys.path.insert(0, "/opt/trn_rl_repo")

N_CORES = 8
B, C, T = 8, 64, 256
EXP = 32
KT = T // 128  # 2 k-tiles over the feature dim
EG = 4         # experts per grouped mask-multiply
INL = 8        # experts masked inline (cover the round-trip latency)
WE_SCALE = 128.0

_CACHE: dict = {}

# fp32 blob layouts: (name, partitions, shape). cols = prod(shape[1:]).
BLOB_A1_SPEC = [
    ("xtl", 128, (128, KT, C)), ("wqt", 128, (128, KT, T)),
    ("bqp", 128, (128, KT)),
]
BLOB_A1B_SPEC = [
    ("xtr", 128, (128, KT, C)), ("wkt", 128, (128, KT, T)),
    ("bkp", 128, (128, KT)),
]
BLOB_A2_SPEC = [
    ("wvt", 128, (128, KT, T)), ("bv", 64, (64, T)),
]
BLOB_B_SPEC = [
    ("wpt", 128, (128, KT, T)), ("wrt", 128, (128, 2 * KT, EXP)),
    ("ident", 128, (128, 128)), ("sel", 2, (2, 2, 128)),
    ("bpp", 128, (128, KT)),
    ("agp", 128, (128, 2, KT)),          # [p, side, kt] LN1 gamma per-partition
    ("ablx", 128, (128, KT, 2, C)),      # (beta + x).T  [p, kt, side, C]
    ("mgall", 128, (128, T)),            # LN2 gamma rows, (c, side) interleaved
    ("mball", 128, (128, T)),            # LN2 beta rows, (c, side) interleaved
    ("cent", 32, (32, C)),
    ("brp", 32, (32, 1)),
    ("eiota", 64, (64, 1)),
    ("identh", 64, (64, C // 2)),
    ("eior8", 128, (128, INL)),          # iota 0..INL-1 on all partitions
    ("onesc", 128, (128, 1)),            # ones column for stats matmuls
    ("onesr", 1, (1, 128)),              # ones row for coef replication
]


def _blob_layout():
    off = {}
    na1 = 0
    for name, parts, shape in BLOB_A1_SPEC:
        cols = int(np.prod(shape[1:]))
        off[name] = (na1, parts, shape)
        na1 += cols
    na1b = 0
    for name, parts, shape in BLOB_A1B_SPEC:
        cols = int(np.prod(shape[1:]))
        off[name] = (na1b, parts, shape)
        na1b += cols
    na2 = 0
    for name, parts, shape in BLOB_A2_SPEC:
        cols = int(np.prod(shape[1:]))
        off[name] = (na2, parts, shape)
        na2 += cols
    nb = 0
    for name, parts, shape in BLOB_B_SPEC:
        cols = int(np.prod(shape[1:]))
        off[name] = (nb, parts, shape)
        nb += cols
    return off, na1, na1b, na2, nb


BLOB_OFF, NA1_COLS, NA1B_COLS, NA2_COLS, NB_COLS = _blob_layout()


def _build():
    import concourse.bass as bass
    import concourse.mybir as mybir
    import concourse.tile as tile
    from concourse import bacc
    from contextlib import ExitStack

    dt = mybir.dt
    f32, f16, f8 = dt.float32, dt.float16, dt.float8e3
    AF = mybir.ActivationFunctionType
    OP = mybir.AluOpType

    nc = bacc.Bacc("TRN2", target_bir_lowering=False, debug=False,
                   num_devices=N_CORES)

    def inp(name, shape, d=f32):
        return nc.dram_tensor(name, list(shape), d, kind="ExternalInput")

    blobA1_d = inp("blobA1", (128, NA1_COLS))
    blobA1b_d = inp("blobA1b", (128, NA1B_COLS))
    blobA2_d = inp("blobA2", (128, NA2_COLS))
    blobB_d = inp("blobB", (128, NB_COLS))
    weh_d = inp("weh", (128, C, KT, T), f8)   # We[e].T tiled [p, e, kt, u], x128
    beh_d = inp("beh", (C, T), f16)           # be natural fp16, x128

    o2_d = nc.dram_tensor("o2", [128, T], f16, kind="ExternalOutput")

    with tile.TileContext(nc) as tc, ExitStack() as ctx:
        cst = ctx.enter_context(tc.tile_pool(name="cst", bufs=1))
        wk = ctx.enter_context(tc.tile_pool(name="wk", bufs=2))
        sm = ctx.enter_context(tc.tile_pool(name="sm", bufs=2))
        asc_p = ctx.enter_context(tc.tile_pool(name="asc", bufs=4))
        ps = ctx.enter_context(tc.tile_pool(name="ps", bufs=4, space="PSUM"))
        ps_moe_p = ctx.enter_context(tc.tile_pool(name="psmoe", bufs=1, space="PSUM"))
        warm_p = ctx.enter_context(tc.tile_pool(name="warm", bufs=1, space="PSUM"))

        # ---- warm-up source: very first DVE op, so the PE warm-up can
        # start the moment the start barrier clears ----
        wsrc = cst.tile([128, 512], f16, tag="wsrc")
        nc.vector.memset(wsrc, 0.5)

        # ---- loads: attention-critical first ----
        blobA1 = cst.tile([128, NA1_COLS], f32, tag="blobA1")
        nc.sync.dma_start(out=blobA1, in_=blobA1_d.ap())
        blobA1b = cst.tile([128, NA1B_COLS], f32, tag="blobA1b")
        nc.scalar.dma_start(out=blobA1b, in_=blobA1b_d.ap())
        blobA2 = cst.tile([128, NA2_COLS], f32, tag="blobA2")
        nc.sync.dma_start(out=blobA2, in_=blobA2_d.ap())
        blobB = cst.tile([128, NB_COLS], f32, tag="blobB")
        nc.sync.dma_start(out=blobB, in_=blobB_d.ap())
        we_sb = cst.tile([128, C, KT, T], f8, tag="weh")
        wea = weh_d.ap()
        for ch in range(4):
            nc.sync.dma_start(out=we_sb[:, ch * 16:(ch + 1) * 16],
                              in_=wea[:, ch * 16:(ch + 1) * 16])
        beh = cst.tile([C, T], f16, tag="beh")
        nc.scalar.dma_start(out=beh, in_=beh_d.ap())

        def bview(blob, name):
            off, parts, shape = BLOB_OFF[name]
            cols = 1
            for s in shape[1:]:
                cols *= s
            v = blob[0:parts, off:off + cols]
            if len(shape) == 3:
                v = v.rearrange("p (a b) -> p a b", a=shape[1])
            elif len(shape) == 4:
                v = v.rearrange("p (a b c) -> p a b c", a=shape[1], b=shape[2])
            return v

        xtl = bview(blobA1, "xtl")
        wqt = bview(blobA1, "wqt")
        bqp = bview(blobA1, "bqp")
        xtr = bview(blobA1b, "xtr")
        wkt = bview(blobA1b, "wkt")
        bkp = bview(blobA1b, "bkp")
        wvt = bview(blobA2, "wvt")
        bv = bview(blobA2, "bv")
        wpt = bview(blobB, "wpt")
        wrt = bview(blobB, "wrt")
        ident = bview(blobB, "ident")
        sel = bview(blobB, "sel")
        bpp = bview(blobB, "bpp")
        agp = bview(blobB, "agp")
        ablx = bview(blobB, "ablx")
        mgall = bview(blobB, "mgall")
        mball = bview(blobB, "mball")
        cent = bview(blobB, "cent")
        brp = bview(blobB, "brp")
        eiota = bview(blobB, "eiota")
        identh_f32 = bview(blobB, "identh")
        identh = identh_f32.bitcast(mybir.dt.float16)
        eior8 = bview(blobB, "eior8")
        onesc = bview(blobB, "onesc")
        onesr = bview(blobB, "onesr")

        eps1_t = cst.tile([128, 1], f32, tag="eps1")
        nc.vector.memset(eps1_t, 1e-5)
        eps2_t = cst.tile([128, 1], f32, tag="eps2")
        nc.vector.memset(eps2_t, 1e-5 * WE_SCALE * WE_SCALE)
        # ACT table preloads
        wact = cst.tile([1, 32], f32, tag="wact")
        nc.vector.memset(wact, 1.0)
        nc.scalar.activation(out=wact, in_=wact, func=AF.Sqrt)
        nc.scalar.activation(out=wact, in_=wact, func=AF.Exp)

        def warm(n):
            pwx = warm_p.tile([128, 512], f32, tag="warm")
            for _ in range(n):
                nc.tensor.matmul(pwx, wsrc[:, 0:128], wsrc,
                                 start=True, stop=True, skip_group_check=True)

        # ---- attention ----
        # xdt early: DVE computes it while PE runs q/k
        xdt = wk.tile([128, KT, C], f32, tag="xdt")
        nc.vector.tensor_sub(xdt, xtl, xtr)
        # q.T (prescaled by 1/16 incl. bias), k.T  [u(128), kt, c]
        # bias adds on the ACT engine (DVE stays free)
        qt = wk.tile([128, KT, C], f32, tag="qt")
        ktl = wk.tile([128, KT, C], f32, tag="ktl")
        for (src, w, bias, dst, s2) in [(xtl, wqt, bqp, qt, 1.0 / 16.0),
                                        (xtr, wkt, bkp, ktl, 1.0)]:
            for ut in range(KT):
                p = ps.tile([128, C], f32, tag="ps")
                for kt in range(KT):
                    nc.tensor.matmul(p, w[:, kt, ut * 128:(ut + 1) * 128],
                                     src[:, kt], start=(kt == 0), stop=(kt == KT - 1))
                nc.scalar.activation(out=dst[:, ut], in_=p, func=AF.Identity,
                                     bias=bias[:, ut:ut + 1], scale=s2)

        # ---- energy + softmax (no max-subtraction; |e/16| < ~1) ----
        pe_ = ps.tile([C, C], f32, tag="ps")
        for ut in range(KT):
            nc.tensor.matmul(pe_, qt[:, ut], ktl[:, ut],
                             start=(ut == 0), stop=(ut == KT - 1))
        warm(2)
        attn = wk.tile([C, C], f32, tag="attn")
        rowsum = sm.tile([C, 1], f32, tag="rowsum")
        nc.scalar.activation(out=attn, in_=pe_, func=AF.Exp, accum_out=rowsum)

        # ---- v = (x_l - x_r) @ Wv.T + bv: overlaps the softmax chain ----
        pv = ps.tile([C, T], f32, tag="ps")
        for kt in range(KT):
            nc.tensor.matmul(pv, xdt[:, kt], wvt[:, kt],
                             start=(kt == 0), stop=(kt == KT - 1))
        v_sb = wk.tile([C, T], f32, tag="v")
        nc.vector.tensor_tensor(out=v_sb, in0=pv, in1=bv, op=OP.add)

        rinv = sm.tile([C, 1], f32, tag="rinv")
        nc.vector.reciprocal(rinv, rowsum)
        nc.vector.tensor_scalar_mul(attn, attn, rinv)

        # ---- attn.T ----
        pat = ps.tile([C, C], f32, tag="ps")
        nc.tensor.transpose(pat, attn, ident[0:C, 0:C])
        attnT = wk.tile([C, C], f32, tag="attnT")
        nc.vector.tensor_copy(attnT, pat)

        # ---- out_l.T / out_r.T  [u, c] ----
        oLT = wk.tile([128, KT, C], f32, tag="oLT")
        oRT = wk.tile([128, KT, C], f32, tag="oRT")
        for ut in range(KT):
            pl = ps.tile([128, C], f32, tag="ps")
            nc.tensor.matmul(pl, v_sb[:, ut * 128:(ut + 1) * 128], attnT,
                             start=True, stop=True)
            nc.vector.tensor_copy(oLT[:, ut], pl)
        warm(3)
        for ut in range(KT):
            pr = ps.tile([128, C], f32, tag="ps")
            nc.tensor.matmul(pr, v_sb[:, ut * 128:(ut + 1) * 128], attn,
                             start=True, stop=True)
            nc.scalar.activation(out=oRT[:, ut], in_=pr, func=AF.Identity)

        # ---- proj (transposed): pT [p, kt, side, C] = (Wp @ out.T) + bp ----
        pT = wk.tile([128, KT, 2, C], f32, tag="pT")
        for s, oT in ((0, oLT), (1, oRT)):
            for ut in range(KT):
                pp = ps.tile([128, C], f32, tag="ps")
                for kt in range(KT):
                    nc.tensor.matmul(pp, wpt[:, kt, ut * 128:(ut + 1) * 128],
                                     oT[:, kt], start=(kt == 0), stop=(kt == KT - 1))
                if s == 0:
                    nc.scalar.activation(out=pT[:, ut, s], in_=pp,
                                         func=AF.Identity,
                                         bias=bpp[:, ut:ut + 1])
                else:
                    nc.vector.tensor_scalar(out=pT[:, ut, s], in0=pp,
                                            scalar1=bpp[:, ut:ut + 1],
                                            scalar2=None, op0=OP.add)

        # ---- LN1 in transposed space: per-column stats via PE ones-matmuls --
        sq = wk.tile([128, KT, 2, C], f32, tag="sq")
        nc.vector.tensor_mul(sq, pT, pT)
        # column-form stats: psum [128(s,c), 2] via pT/sq as stationary
        pstat = ps.tile([128, 2], f32, tag="ps")
        for kt in range(KT):
            nc.tensor.matmul(pstat[:, 0:1], pT[:, kt], onesc,
                             start=(kt == 0), stop=(kt == KT - 1))
        for kt in range(KT):
            nc.tensor.matmul(pstat[:, 1:2], sq[:, kt], onesc,
                             start=(kt == 0), stop=(kt == KT - 1))
        warm(4)
        # coefs: a = rstd, b = mean * rstd   (LN = pT*a - b, then *g + ablx)
        msv = sm.tile([128, 2], f32, tag="msv")
        nc.vector.tensor_scalar(out=msv, in0=pstat, scalar1=1.0 / T,
                                scalar2=None, op0=OP.mult)
        m1 = msv[:, 0:1]
        var = sm.tile([128, 1], f32, tag="var")
        nc.vector.tensor_mul(var, m1, m1)
        nc.vector.tensor_sub(var, msv[:, 1:2], var)
        coefc = sm.tile([128, 2], f32, tag="coefc")
        nc.scalar.activation(out=coefc[:, 0:1], in_=var, func=AF.Sqrt,
                             bias=eps1_t)
        nc.vector.reciprocal(coefc[:, 0:1], coefc[:, 0:1])
        nc.vector.tensor_mul(coefc[:, 1:2], m1, coefc[:, 0:1])
        # replicate the two coef columns to all partitions: transpose + 2 MMs
        ptc = ps.tile([2, 128], f32, tag="ps")
        nc.tensor.transpose(ptc, coefc, ident)
        coefr = sm.tile([2, 128], f32, tag="coefr")
        nc.scalar.activation(out=coefr, in_=ptc, func=AF.Identity)
        prep = ps.tile([128, 2, 128], f32, tag="ps")
        for j in range(2):
            nc.tensor.matmul(prep[:, j], sel[:, j], coefr,
                             start=True, stop=True)
        warm(3)
        aB = prep[:, 0].rearrange("p (s c) -> p s c", s=2).unsqueeze(1) \
            .broadcast_to((128, KT, 2, C))
        bB = prep[:, 1].rearrange("p (s c) -> p s c", s=2).unsqueeze(1) \
            .broadcast_to((128, KT, 2, C))
        nc.vector.tensor_mul(pT, pT, aB)
        nc.vector.tensor_sub(pT, pT, bB)
        for s in range(2):
            for kt in range(KT):
                if s == 0:
                    nc.scalar.activation(out=pT[:, kt, s], in_=pT[:, kt, s],
                                         func=AF.Identity,
                                         scale=agp[:, s, kt:kt + 1])
                else:
                    nc.vector.tensor_scalar(out=pT[:, kt, s], in0=pT[:, kt, s],
                                            scalar1=agp[:, s, kt:kt + 1],
                                            scalar2=None, op0=OP.mult)
        nc.vector.tensor_add(pT, pT, ablx)
        # pT now holds OUT.T (LN1 output + residual), both sides.

        # ---- router (row-norms dropped; picks invariant to row scale) ----
        pxp = ps.tile([EXP, C], f32, tag="ps")
        j = 0
        for s in range(2):
            for kt in range(KT):
                nc.tensor.matmul(pxp, wrt[:, j], pT[:, kt, s],
                                 start=(j == 0), stop=(j == 2 * KT - 1))
                j += 1
        xpT = wk.tile([EXP, C], f32, tag="xpT")
        nc.scalar.activation(out=xpT, in_=pxp, func=AF.Identity, bias=brp)
        psim = ps.tile([C, C], f32, tag="ps")
        nc.tensor.matmul(psim, xpT, cent, start=True, stop=True)

        warm(3)  # PE filler while DVE runs top-k

        mx8 = sm.tile([C, 8], f32, tag="mx8")
        nc.vector.max(out=mx8, in_=psim)
        idx8 = sm.tile([C, 8], mybir.dt.uint32, tag="idx8")
        nc.vector.max_index(out=idx8, in_max=mx8, in_values=psim)

        # ---- replicate topi rows across partitions: one matmul per k with a
        # stride-0 stationary (topih column broadcast over 128 cols) ----
        topih = sm.tile([C, 2], f16, tag="topih")
        nc.vector.tensor_copy(topih, idx8[:, 0:2])

        # fp16 copy, kt innermost: oAll [p, side, C, kt]
        oAll = wk.tile([128, 2, C, KT], f16, tag="oAll")
        nc.vector.tensor_copy(oAll, pT[:].transpose([0, 2, 3, 1]))
        ttrep_ps = []
        for k in range(2):
            pr = ps.tile([128, C], f32, tag="ps")
            nc.tensor.matmul(pr,
                             topih[:, k:k + 1].broadcast_to((C, 128)),
                             identh, start=True, stop=True)
            ttrep_ps.append(pr)

        warm(2)

        # ---- RT[e, c] one-hot-sum mask matrix ----
        RT = wk.tile([C, C], f32, tag="RT")
        RT1 = sm.tile([C, C], f32, tag="RT1")
        nc.vector.tensor_scalar(out=RT, in0=ttrep_ps[0][0:C], scalar1=eiota,
                                scalar2=None, op0=OP.is_equal)
        nc.vector.tensor_scalar(out=RT1, in0=ttrep_ps[1][0:C], scalar1=eiota,
                                scalar2=None, op0=OP.is_equal)
        RTh = wk.tile([C, C], f16, tag="RTh")
        nc.vector.tensor_tensor(out=RTh, in0=RT, in1=RT1, op=OP.add)
        # bias-matmul stationary: RT duplicated to (side, c) columns, fp16
        RTb = wk.tile([C, 2, C], f16, tag="RTb")
        nc.vector.tensor_copy(RTb, RTh.unsqueeze(1).broadcast_to((C, 2, C)))

        # ---- replicate RTh rows across partitions via a DRAM round-trip
        # (HWDGE write on the scalar ring; SWDGE replicated reads) ----
        dram = ctx.enter_context(tc.tile_pool(name="dram", bufs=1, space="DRAM"))
        rtd = dram.tile([C, C], f16)
        nc.scalar.dma_start(out=rtd[:], in_=RTh)
        rrep = wk.tile([128, C, C], f16, tag="rrep")
        rsrc = rtd[:]
        for (c0, c1) in ((INL, 12), (12, 24), (24, 40), (40, C)):
            src_ap = bass.AP(tensor=rsrc.tensor, offset=rsrc.offset + c0 * C,
                             ap=[[0, 128], [C, c1 - c0], [1, C]])
            nc.scalar.dma_start(out=rrep[:, c0:c1], in_=src_ap)

        # ---- inline masks for experts 0..INL-1 (batched is_equal, in
        # halves so the first expert matmuls start sooner) ----
        allm = wk.tile([128, INL, C], f32, tag="allm")
        allm1 = wk.tile([128, INL, C], f32, tag="allm1")
        H = INL // 2
        for h0 in (0, H):
            hs = slice(h0, h0 + H)
            nc.vector.tensor_tensor(
                out=allm[:, hs],
                in0=ttrep_ps[0].unsqueeze(1).broadcast_to((128, H, C)),
                in1=eior8[:, hs].unsqueeze(2).broadcast_to((128, H, C)),
                op=OP.is_equal)
            nc.vector.tensor_tensor(
                out=allm1[:, hs],
                in0=ttrep_ps[1].unsqueeze(1).broadcast_to((128, H, C)),
                in1=eior8[:, hs].unsqueeze(2).broadcast_to((128, H, C)),
                op=OP.is_equal)
            nc.vector.tensor_add(allm[:, hs], allm[:, hs], allm1[:, hs])

        # ---- expert stage ----
        outnat = wk.tile([128, T], f32, tag="outnat")
        ps_moe = ps_moe_p.tile([128, T], f32, tag="psmoe")
        nc.tensor.matmul(ps_moe, RTb, beh,
                         start=True, stop=False, skip_group_check=True)
        warm(5)
        oPair = oAll[:].bitcast(mybir.dt.float32).squeeze(3)  # [p, side, C]
        groups = [(0, 2), (2, 4), (4, 8)] + \
            [(e, e + EG) for e in range(INL, C, EG)]
        for (e0, e1) in groups:
            ng = e1 - e0
            asch = asc_p.tile([128, ng, 2, C, KT], f16, tag=f"asch{ng}")
            aschP = asch[:].bitcast(mybir.dt.float32).squeeze(4)
            in0 = oPair.unsqueeze(1).broadcast_to((128, ng, 2, C))
            if e0 < INL:
                in1 = allm[:, e0:e1].unsqueeze(2) \
                    .broadcast_to((128, ng, 2, C))
            else:
                in1 = rrep[:, e0:e1].unsqueeze(2) \
                    .broadcast_to((128, ng, 2, C))
            if e0 == 16:
                for s in range(2):
                    for kt in range(KT):
                        ptn = ps.tile([C, 128], f32, tag="ps")
                        nc.tensor.transpose(ptn, pT[:, kt, s], ident)
                        nc.vector.tensor_copy(
                            outnat[s * C:(s + 1) * C,
                                   kt * 128:(kt + 1) * 128], ptn)
            nc.vector.tensor_tensor(out=aschP, in0=in0, in1=in1, op=OP.mult)
            for i in range(ng):
                for kt in range(KT):
                    st = asch[:, i, :, :, kt]
                    nc.tensor.matmul(ps_moe, st, we_sb[:, e0 + i, kt],
                                     start=False,
                                     stop=(e1 >= C and i == ng - 1
                                           and kt == KT - 1),
                                     skip_group_check=True)

        # ---- final LN2 + residual, both sides at once ----
        obl = wk.tile([128, T], f32, tag="obl")
        nc.vector.tensor_add(obl, outnat, mball)
        stats = sm.tile([128, 6], f32, tag="stats2")
        nc.vector.bn_stats(out=stats, in_=ps_moe)
        mv = sm.tile([128, 2], f32, tag="mv2")
        nc.vector.bn_aggr(out=mv, in_=stats)
        rstd2 = sm.tile([128, 1], f32, tag="rstd2")
        nc.scalar.activation(out=rstd2, in_=mv[:, 1:2], func=AF.Sqrt,
                             bias=eps2_t)
        nc.vector.reciprocal(rstd2, rstd2)
        nb2 = sm.tile([128, 1], f32, tag="nb2")
        nc.vector.tensor_scalar(out=nb2, in0=mv[:, 0:1], scalar1=rstd2,
                                scalar2=-1.0, op0=OP.mult, op1=OP.mult)
        o2 = wk.tile([128, T], f16, tag="o2")
        o2a = o2_d.ap()
        for h in range(2):
            cs = slice(h * 128, (h + 1) * 128)
            nc.scalar.activation(out=o2[:, cs], in_=ps_moe[:, cs],
                                 func=AF.Identity, scale=rstd2, bias=nb2)
            nc.vector.tensor_tensor(out=o2[:, cs], in0=o2[:, cs],
                                    in1=mgall[:, cs], op=OP.mult)
            nc.vector.tensor_tensor(out=o2[:, cs], in0=o2[:, cs],
                                    in1=obl[:, cs], op=OP.add)
            nc.sync.dma_start(out=o2a[:, cs], in_=o2[:, cs])

    nc.compile()
    return nc


def _tile_t(w):
    # (T_in, N) -> [128, T_in//128, N] partition-tiled
    t_in, n = w.shape
    return np.ascontiguousarray(w.reshape(t_in // 128, 128, n).transpose(1, 0, 2))


def _interleave_rows(a, b):
    # [C, T], [C, T] -> [2C, T] with rows (c, side)-interleaved
    out = np.empty((2 * C, a.shape[1]), a.dtype)
    out[0::2] = a
    out[1::2] = b
    return out


def _prep_in_maps(inputs):
    f = np.float32
    import ml_dtypes
    x_l, x_r = inputs["x_l"], inputs["x_r"]

    cen = np.asarray(inputs["centers"], f)
    cenn = cen / np.maximum(np.linalg.norm(cen, axis=-1, keepdims=True), 1e-12)
    sel = np.zeros((2, 2, 128), f)
    sel[0, 0, :] = 1.0
    sel[1, 1, :] = 1.0

    def perp(b):  # (T,) -> [128, KT] per-partition layout
        return np.asarray(b, f).reshape(KT, 128).T

    ag = np.stack([np.asarray(inputs["ag_l"], f), np.asarray(inputs["ag_r"], f)])
    agp = ag.reshape(2, KT, 128).transpose(2, 0, 1)  # [p, side, kt]

    mgall = np.concatenate([
        np.repeat(np.asarray(inputs["mg_l"], f).reshape(1, T), C, axis=0),
        np.repeat(np.asarray(inputs["mg_r"], f).reshape(1, T), C, axis=0)])
    mball = np.concatenate([
        np.repeat(np.asarray(inputs["mb_l"], f).reshape(1, T), C, axis=0),
        np.repeat(np.asarray(inputs["mb_r"], f).reshape(1, T), C, axis=0)])

    arrs = {
        "wqt": _tile_t(np.asarray(inputs["Wq"], f).T),
        "wkt": _tile_t(np.asarray(inputs["Wk"], f).T),
        "wvt": _tile_t(np.asarray(inputs["Wv"], f).T),
        "wpt": _tile_t(np.asarray(inputs["Wp"], f).T),
        "bqp": perp(np.asarray(inputs["bq"], f) / 16.0),
        "bkp": perp(inputs["bk"]),
        "bpp": perp(inputs["bp"]),
        "agp": agp,
        "wrt": _tile_t(np.asarray(inputs["Wr"], f).T),
        "brp": np.asarray(inputs["br"], f).reshape(EXP, 1),
        "cent": np.ascontiguousarray(cenn.T),
        "ident": np.eye(128, dtype=f),
        "eiota": np.arange(C, dtype=f).reshape(C, 1),
        "identh": np.eye(C, dtype=np.float16).view(f),
        "eior8": np.repeat(np.arange(INL, dtype=f).reshape(1, INL), 128, axis=0),
        "onesc": np.ones((128, 1), f),
        "onesr": np.ones((1, 128), f),
        "sel": sel,
        "bv": np.repeat(np.asarray(inputs["bv"], f).reshape(1, T), C, axis=0),
        "mgall": mgall, "mball": mball,
    }
    We = np.asarray(inputs["We"], f)
    WeTh = np.ascontiguousarray(
        (We * WE_SCALE).transpose(0, 2, 1).reshape(C, KT, 128, T)
        .transpose(2, 0, 1, 3)
    ).astype(ml_dtypes.float8_e3m4)
    beh = (np.asarray(inputs["be"], f) * WE_SCALE).astype(np.float16)

    def pack(spec, ncols, extra):
        blob = np.zeros((128, ncols), f)
        for name, parts, shape in spec:
            off, _, _ = BLOB_OFF[name]
            cols = int(np.prod(shape[1:]))
            a = extra[name] if name in extra else arrs[name]
            blob[0:parts, off:off + cols] = np.asarray(a, f).reshape(parts, cols)
        return blob

    ab_l = np.asarray(inputs["ab_l"], f)
    ab_r = np.asarray(inputs["ab_r"], f)
    in_maps = []
    for b in range(N_CORES):
        xlb = np.asarray(x_l[b], f)
        xrb = np.asarray(x_r[b], f)
        xtl = _tile_t(np.ascontiguousarray(xlb.T))
        xtr = _tile_t(np.ascontiguousarray(xrb.T))
        blobA1 = pack(BLOB_A1_SPEC, NA1_COLS, {"xtl": xtl})
        blobA1b = pack(BLOB_A1B_SPEC, NA1B_COLS, {"xtr": xtr})
        blobA2 = pack(BLOB_A2_SPEC, NA2_COLS, {})
        # (beta + x).T tiled: [p, kt, side, C]
        ablx = np.stack([_tile_t(np.ascontiguousarray((ab_l + xlb).T)),
                         _tile_t(np.ascontiguousarray((ab_r + xrb).T))], axis=2)
        blobB = pack(BLOB_B_SPEC, NB_COLS, {"ablx": ablx})
        in_maps.append({"blobA1": blobA1, "blobA1b": blobA1b,
                        "blobA2": blobA2, "blobB": blobB,
                        "weh": WeTh, "beh": beh})
    return in_maps


def kernel(**inputs) -> np.ndarray:
    from concourse.bass_utils import run_bass_kernel_spmd

    if "nc" not in _CACHE:
        _CACHE["nc"] = _build()
    nc = _CACHE["nc"]
    in_maps = _prep_in_maps(inputs)
    res = run_bass_kernel_spmd(nc, in_maps, list(range(N_CORES)))
    _CACHE["exec_time_ns"] = res.exec_time_ns
    o2 = np.stack([res.results[b]["o2"] for b in range(N_CORES)])  # [B,128,T]
    out_l2 = o2[:, 0:C]
    out_r2 = o2[:, C:2 * C]
    return np.stack([out_l2, out_r2]).astype(np.float32)
